# revision 7
# baseline (speedup 1.0000x reference)
"""Trainium2 Bass kernel: elementary CA (rule 110) via PE band-matmul.

Layout (per core): 2 batch rows; each row's 32768 cells live in a
"transposed" tile S[p, f] = cell w = (f-G)*128 + p (mod W), i.e. the
3-cell stencil runs along the PARTITION axis.  One CA step is then:

    x  = -0.5*l + c - 2*r     (PE: band matmul + 2 wrap-fix matmuls,
                               accumulated in PSUM; exact dyadic fp32)
    ns = ((x >= -1) == c)     (single DVE scalar_tensor_tensor op;
                               exact rule 110 on all 8 (l,c,r)
                               patterns - verified by enumeration)

The circular wrap of each row is in the FREE dim: G ghost columns on
each side evolve along with the interior and are rebuilt every G steps
by two small ACT copies (no cross-partition permutes needed); the
seed's ghosts are pre-padded by the host so startup skips the refresh.

The two rows are fully independent chains, so PE work of one row
overlaps DVE work of the other.  Outputs stage in SBUF and flush to
HBM in kf-step blocks of 1KB-contiguous lines; the host transposes
(f,p) -> w order afterwards (pure marshalling; not on the device).
"""

import os
import sys
import types
import numpy as np

P = 128
N_CORES = 8
B_FULL = 16
ROWS = B_FULL // N_CORES
W = 32768
C = W // P                  # 256 free columns per row
RULE110 = (0, 1, 1, 1, 0, 1, 1, 0)

_PROGRAM_CACHE = {}
_LAST_RESULTS = None


def _install_ntff_shim():
    try:
        from antenv.axon_hooks import get_axon_ntff_profile_hook  # noqa: F401
        return
    except ImportError:
        pass
    try:
        import antenv
        import trn_agent_boot.trn_boot as tb
        mod = types.ModuleType("antenv.axon_hooks")
        _hook = [None]
        mod.set_axon_ntff_profile_hook = lambda h: _hook.__setitem__(0, h)
        mod.get_axon_ntff_profile_hook = lambda: _hook[0]
        sys.modules["antenv.axon_hooks"] = mod
        antenv.axon_hooks = mod
        mod.set_axon_ntff_profile_hook(
            tb._ntff_profile_via_ctypes("/opt/axon/libaxon_pjrt.so")
        )
    except Exception:
        pass


def _build_program(steps, ghost=16, kf=8):
    import concourse.bacc as bacc
    from concourse import mybir, tile
    from concourse.alu_op_type import AluOpType as aop

    f32 = mybir.dt.float32
    bf16 = mybir.dt.bfloat16
    G = ghost
    F = C + 2 * G               # tile width per row

    nc = bacc.Bacc("TRN2", target_bir_lowering=False, debug=False,
                   num_devices=N_CORES)
    x = nc.dram_tensor("x", [P, ROWS * F], bf16, kind="ExternalInput").ap()
    wmat = nc.dram_tensor("wmat", [3, P, P], bf16, kind="ExternalInput").ap()
    y = nc.dram_tensor("y", [ROWS, steps + 1, P, C], bf16,
                       kind="ExternalOutput").ap()

    with tile.TileContext(nc) as tc:
        with (
            tc.tile_pool(name="const", bufs=1) as const_pool,
            tc.tile_pool(name="stage", bufs=3) as stage_pool,
            tc.tile_pool(name="ps", bufs=4, space="PSUM") as ps_pool,
        ):
            wb = const_pool.tile([P, P], bf16, tag="wb")
            wl = const_pool.tile([P, P], bf16, tag="wl")
            wr = const_pool.tile([P, P], bf16, tag="wr")
            # weights on the ACT hwdge queue, seeds on sync: the two
            # input streams' descriptor processing runs in parallel
            nc.scalar.dma_start(out=wb[:, :], in_=wmat[0])
            nc.scalar.dma_start(out=wl[:, :], in_=wmat[1])
            nc.scalar.dma_start(out=wr[:, :], in_=wmat[2])

            def refresh_ghosts(s):
                # s: [P, F]; interior [G, F-G) valid. Circular wrap:
                # left ghosts [0,G) <- interior cols [C, C+G)
                # right ghosts [F-G, F) <- interior cols [G, 2G)
                nc.scalar.copy(out=s[:, 0:G], in_=s[:, C:C + G])
                nc.scalar.copy(out=s[:, F - G:F], in_=s[:, G:2 * G])

            def emit_step_mm(ss, pss, m):
                # per row: x into ps cols [m+1, F-m-1); grouped by weight
                # matrix so the stationary tensor reloads less often
                lo, hi = m + 1, F - m - 1
                for s, ps in zip(ss, pss):
                    nc.tensor.matmul(ps[:, lo:hi], wb[:, :], s[:, lo:hi],
                                     start=True, stop=False)
                for s, ps in zip(ss, pss):
                    nc.tensor.matmul(ps[:, lo:hi], wl[:, :],
                                     s[:, lo - 1:hi - 1],
                                     start=False, stop=False)
                for s, ps in zip(ss, pss):
                    nc.tensor.matmul(ps[:, lo:hi], wr[:, :],
                                     s[:, lo + 1:hi + 1],
                                     start=False, stop=True)

            def emit_map(s, ps, out, m):
                # ns = ((x >= -1) == c), x = -0.5*l + c - 2*r  (exact rule 110)
                lo, hi = m + 1, F - m - 1
                nc.vector.scalar_tensor_tensor(
                    out[:, lo:hi], ps[:, lo:hi], -1.0, s[:, lo:hi],
                    aop.is_ge, aop.is_equal)

            # Seed: host pre-pads ghosts and interleaves both rows per
            # partition -> one DMA, 128 double-width descriptors, both
            # rows land together (halves the sync-queue startup serial).
            seed = const_pool.tile([P, ROWS, F], bf16, tag="seed")
            nc.sync.dma_start(
                out=seed[:, :, :],
                in_=x.rearrange("p (r f) -> p r f", r=ROWS))
            seeds = [seed[:, r, :] for r in range(ROWS)]

            prev = list(seeds)          # current state tile per row
            n = 0
            while n < steps:
                kk = min(kf, steps - n)
                st = stage_pool.tile([P, kf * ROWS * F], bf16, tag="st")
                st4 = st[:, :].rearrange("p (k r f) -> p k r f", k=kf, r=ROWS)
                for k in range(kk):
                    m = (n + k) % G
                    pss = [ps_pool.tile([P, F], f32, tag=f"ps{r}",
                                        name=f"ps{r}")
                           for r in range(ROWS)]
                    # group matmuls by weight matrix across rows to halve
                    # stationary-weight reloads
                    emit_step_mm(prev, pss, m)
                    for r in range(ROWS):
                        emit_map(prev[r], pss[r], st4[:, k, r], m)
                        prev[r] = st4[:, k, r]
                        if (n + k + 1) % G == 0 and n + k + 1 < steps:
                            refresh_ghosts(prev[r])
                for r in range(ROWS):
                    yv = y[r].rearrange("t p j -> p t j")
                    nc.sync.dma_start(
                        out=yv[:, 1 + n:1 + n + kk, :],
                        in_=st4[:, 0:kk, r, G:G + C])
                n += kk

            # y[:, 0] = seed (flush last; only needs to land by kernel end)
            for r in range(ROWS):
                yv = y[r].rearrange("t p j -> p t j")
                nc.sync.dma_start(out=yv[:, 0:1, :],
                                  in_=seeds[r][:, G:G + C].rearrange(
                                      "p (t j) -> p t j", t=1))

    nc.compile()
    return nc


def _weight_mats():
    # out[i,f] = sum_k lhsT[k,i] * rhs[k,f]; x = -0.5*l + 1*c - 2*r
    AL, AC, AR = -0.5, 1.0, -2.0
    wb = np.zeros((P, P), dtype=np.float32)
    wl = np.zeros((P, P), dtype=np.float32)
    wr = np.zeros((P, P), dtype=np.float32)
    for i in range(P):
        if i - 1 >= 0:
            wb[i - 1, i] = AL       # l = s[p-1]
        wb[i, i] = AC               # c
        if i + 1 < P:
            wb[i + 1, i] = AR       # r = s[p+1]
    wl[P - 1, 0] = AL               # out[0] += AL*s[127, f-1]
    wr[0, P - 1] = AR               # out[127] += AR*s[0, f+1]
    return np.stack([wb, wl, wr])


def kernel(state, rule_table, steps):
    global _LAST_RESULTS
    from concourse import bass_utils

    state = np.asarray(state)
    steps = int(steps)
    bits = tuple(int(round(float(v))) for v in np.asarray(rule_table).reshape(-1))
    assert bits == RULE110, f"kernel specialized for rule 110, got {bits}"
    B = state.shape[0]
    assert B == B_FULL and state.shape[-1] == W, state.shape

    if steps not in _PROGRAM_CACHE:
        _PROGRAM_CACHE[steps] = _build_program(steps)
    nc = _PROGRAM_CACHE[steps]

    # host-side marshalling into the transposed layout: cell w -> (p=w%P, j=w//P)
    import ml_dtypes
    bf = ml_dtypes.bfloat16
    xs = state.reshape(B, W)
    x_t = xs.reshape(B, C, P).transpose(0, 2, 1)             # [B, P, C]
    x_t = np.concatenate(
        [x_t[:, :, C - 16:], x_t, x_t[:, :, :16]], axis=2)   # [B, P, F]
    F = C + 32
    x_t = np.ascontiguousarray(
        x_t.reshape(N_CORES, ROWS, P, F).transpose(0, 2, 1, 3)
    ).reshape(N_CORES, P, ROWS * F).astype(bf)
    wmat = _weight_mats().astype(bf)
    in_maps = [{"x": x_t[i], "wmat": wmat} for i in range(N_CORES)]
    trace = os.environ.get("CA_TRACE") == "1"
    if trace:
        _install_ntff_shim()
    res = bass_utils.run_bass_kernel_spmd(nc, in_maps, list(range(N_CORES)),
                                          trace=trace)
    _LAST_RESULTS = res
    out = np.stack([np.asarray(res.results[i]["y"]) for i in range(N_CORES)])
    # [N_CORES, ROWS, T, P, C] -> w order = (j, p) -> j*P + p
    out = out.reshape(B, steps + 1, P, C).transpose(0, 1, 3, 2)
    return np.ascontiguousarray(out).astype(np.float32).reshape(
        B, steps + 1, 1, W)
